# revision 4
# baseline (speedup 1.0000x reference)
"""Trainium2 Bass kernel: 3-layer PyG-style GraphConv stack on a dense
weighted adjacency (nn_NeuroGNN_GNN_GraphConv).

    h_{l+1} = relu( (A^T h_l) @ W_rel_l^T + b_l + h_l @ W_root_l^T )

Distribution (8 NeuronCores): column-shard adj (= shard the OUTPUT node
block): core d holds adj[:, d*B:(d+1)*B] and computes the full
contraction over all 16384 source nodes for its own 2048 output nodes.
The only cross-core traffic is an AllGather of hr = h_l @ W_rel_l^T
([2048, 64] bf16 per core) at each layer boundary.

On-chip layout: h is kept transposed (hT [64 feat, 2048 nodes]) so that
every matmul lands in its natural orientation with zero on-chip
transposes:
  hr[j,f]   (psum) = lhsT(hT[:, jtile]) ^T @ W_relT          -> cast bf16
  aggT[f,i] (psum) = sum_j lhsT(hr[jtile])^T @ adj[jtile, :] (bf16 x bf16)
  aggT     +=        lhsT(W_rootT)^T @ hT[:, isub]           (root term)
  hT'       = relu(aggT + b)                                  (ScalarE)

adj is streamed from HBM as bf16 (host-converted): 64 MiB/core/layer,
which is the roofline term (memory-bound problem).
"""

import numpy as np
import ml_dtypes

import jax
from jax.experimental.shard_map import shard_map
from jax.sharding import Mesh, NamedSharding, PartitionSpec

from concourse import bacc, bass, bass2jax, mybir, tile

F32 = mybir.dt.float32
F32R = mybir.dt.float32r
BF16 = mybir.dt.bfloat16

N_NODES = 16384
F_IN = 256
H = 64
NCORES = 8

# knobs
ADJ_BF16 = True       # stream adj (and hr) as bf16; False -> fp32 (+f32r matmul)
CHUNK = 2             # j-tiles (128 rows) per adj DMA
ADJ_BUFS = 8          # in-flight adj chunks
TRACE = False         # set True (by test.py) to capture NTFF profile
LAST_RESULTS = None   # BassKernelResults of the most recent run


def build_nc(n_nodes=N_NODES, f_in=F_IN, h=H, ncores=NCORES, adj_bf16=ADJ_BF16):
    """Build the SPMD Bass program (one program, runs on all 8 cores)."""
    b = n_nodes // ncores          # own output block width / own j-rows
    n_jt = n_nodes // 128          # global j-tiles
    own_jt = b // 128              # j-tiles in own block
    isub = min(512, b)             # psum bank width for the big matmul
    n_isub = b // isub
    c0 = f_in // 128               # K-tiles over input features (layer 0)

    cdt = BF16 if adj_bf16 else F32
    # for fp32 we bitcast the big matmul operands to float32r (full-rate fp32)
    mmdt = BF16 if adj_bf16 else F32R

    def mm(ap):
        return ap if adj_bf16 else ap.bitcast(F32R)

    nc = bacc.Bacc(
        "TRN2",
        target_bir_lowering=False,
        debug=False,
        num_devices=ncores,
    )

    adjc = nc.dram_tensor("adjc", [n_nodes, b], cdt, kind="ExternalInput").ap()
    xt = nc.dram_tensor("xt", [f_in, b], F32, kind="ExternalInput").ap()
    wrelt = [
        nc.dram_tensor("wrel0t", [f_in, h], F32, kind="ExternalInput").ap(),
        nc.dram_tensor("wrel1t", [h, h], F32, kind="ExternalInput").ap(),
        nc.dram_tensor("wrel2t", [h, h], F32, kind="ExternalInput").ap(),
    ]
    wroott = [
        nc.dram_tensor("wroot0t", [f_in, h], F32, kind="ExternalInput").ap(),
        nc.dram_tensor("wroot1t", [h, h], F32, kind="ExternalInput").ap(),
        nc.dram_tensor("wroot2t", [h, h], F32, kind="ExternalInput").ap(),
    ]
    bias = [
        nc.dram_tensor(f"b{l}", [h, 1], F32, kind="ExternalInput").ap()
        for l in range(3)
    ]
    out = nc.dram_tensor("out", [h, b], F32, kind="ExternalOutput").ap()

    rg = [list(range(ncores))]

    with tile.TileContext(nc) as tc:
        with (
            tc.tile_pool(name="const", bufs=1) as const,
            tc.tile_pool(name="adj", bufs=ADJ_BUFS) as adjp,
            tc.tile_pool(name="hrf", bufs=8) as hrfp,
            tc.tile_pool(name="work", bufs=2) as work,
            tc.tile_pool(name="psb", bufs=n_isub, space="PSUM") as psb,
            tc.tile_pool(name="psh", bufs=2, space="PSUM") as psh,
            tc.tile_pool(name="dram", bufs=1, space="DRAM") as dram,
        ):
            # ---- constants ----
            xt_sb = const.tile([128, c0, b], F32)
            for c in range(c0):
                nc.sync.dma_start(xt_sb[:, c, :], xt[c * 128:(c + 1) * 128, :])

            wrel_sb = []
            wroot_sb = []
            # layer 0 weights have f_in rows -> c0 K-tiles
            w0rel = const.tile([128, c0, h], F32, name="w0rel")
            w0root = const.tile([128, c0, h], F32, name="w0root")
            for c in range(c0):
                nc.sync.dma_start(w0rel[:, c, :], wrelt[0][c * 128:(c + 1) * 128, :])
                nc.sync.dma_start(w0root[:, c, :], wroott[0][c * 128:(c + 1) * 128, :])
            wrel_sb.append(w0rel)
            wroot_sb.append(w0root)
            for l in (1, 2):
                wr = const.tile([h, h], F32, name=f"w{l}rel")
                wo = const.tile([h, h], F32, name=f"w{l}root")
                nc.sync.dma_start(wr[:], wrelt[l][:])
                nc.sync.dma_start(wo[:], wroott[l][:])
                wrel_sb.append(wr)
                wroot_sb.append(wo)
            b_sb = []
            for l in range(3):
                bt = const.tile([h, 1], F32, name=f"b{l}sb")
                nc.sync.dma_start(bt[:], bias[l][:])
                b_sb.append(bt)

            hT = None  # current hidden state, transposed [h, b] (own block)

            for l in range(3):
                # ---- A: local hr tiles + AllGather ----
                hr_own = work.tile([128, own_jt, h], cdt, name="hr_own", tag="hr_own")
                for t in range(own_jt):
                    ps = psh.tile([128, h], F32, name="ps_hr", tag="ps_hr")
                    if l == 0:
                        for c in range(c0):
                            nc.tensor.matmul(
                                ps[:],
                                xt_sb[:, c, t * 128:(t + 1) * 128],
                                wrel_sb[0][:, c, :],
                                start=(c == 0),
                                stop=(c == c0 - 1),
                            )
                    else:
                        nc.tensor.matmul(
                            ps[:],
                            hT[:, t * 128:(t + 1) * 128],
                            wrel_sb[l][:],
                            start=True,
                            stop=True,
                        )
                    nc.vector.tensor_copy(hr_own[:, t, :], ps[:])

                ag_in = dram.tile([b, h], cdt, name=f"ag_in{l}")
                nc.sync.dma_start(
                    ag_in.rearrange("(t p) f -> p t f", p=128), hr_own[:]
                )
                ag_out = dram.tile(
                    [n_nodes, h], cdt, name=f"ag_out{l}", addr_space="Shared"
                )
                nc.gpsimd.collective_compute(
                    "AllGather",
                    mybir.AluOpType.bypass,
                    replica_groups=rg,
                    ins=[ag_in.opt()],
                    outs=[ag_out.opt()],
                )

                # ---- B: big matmul  aggT[f, i] += hr^T adj  (+ root term) ----
                pss = [
                    psb.tile([h, isub], F32, name=f"ps_big{s}", tag="ps_big")
                    for s in range(n_isub)
                ]
                for ch in range(n_jt // CHUNK):
                    adj_t = adjp.tile([128, CHUNK, b], cdt, name="adj_t", tag="adj_t")
                    nc.sync.dma_start(
                        adj_t[:],
                        adjc[ch * CHUNK * 128:(ch + 1) * CHUNK * 128, :].rearrange(
                            "(u p) n -> p u n", p=128
                        ),
                    )
                    for u in range(CHUNK):
                        g = ch * CHUNK + u
                        hrf = hrfp.tile([128, h], cdt, name="hrf", tag="hrf")
                        nc.sync.dma_start(hrf[:], ag_out[g * 128:(g + 1) * 128, :])
                        for s in range(n_isub):
                            nc.tensor.matmul(
                                pss[s][:],
                                mm(hrf[:]),
                                mm(adj_t[:, u, s * isub:(s + 1) * isub]),
                                start=(g == 0),
                                stop=False,
                            )
                # root term, accumulated into the same psum banks
                for s in range(n_isub):
                    if l == 0:
                        for c in range(c0):
                            nc.tensor.matmul(
                                pss[s][:],
                                wroot_sb[0][:, c, :],
                                xt_sb[:, c, s * isub:(s + 1) * isub],
                                start=False,
                                stop=(c == c0 - 1),
                            )
                    else:
                        nc.tensor.matmul(
                            pss[s][:],
                            wroot_sb[l][:],
                            hT[:, s * isub:(s + 1) * isub],
                            start=False,
                            stop=True,
                        )

                # ---- C: hT' = relu(aggT + b) ----
                hT_new = work.tile([h, b], F32, name="hT", tag="hT")
                for s in range(n_isub):
                    nc.scalar.activation(
                        hT_new[:, s * isub:(s + 1) * isub],
                        pss[s][:],
                        mybir.ActivationFunctionType.Relu,
                        bias=b_sb[l][:],
                    )
                hT = hT_new

            nc.sync.dma_start(out[:], hT[:])

    nc.compile()
    return nc


class _Runner:
    """Persistent PJRT executor for one compiled Bass program.

    Mirrors concourse.bass2jax.run_bass_via_pjrt but keeps the jitted
    callable (and optionally device-resident inputs) alive across calls,
    and skips output-buffer donation so the same buffers can be re-executed
    for timing. The kernel writes every element of its outputs, so the
    zero-init the native path provides is not load-bearing here.
    """

    def __init__(self, nc, n_cores):
        bass2jax.install_neuronx_cc_hook()
        assert nc.dbg_addr is None, "build with debug=False"
        self.nc = nc
        self.n_cores = n_cores
        partition_name = (
            nc.partition_id_tensor.name if nc.partition_id_tensor else None
        )
        in_names, out_names, out_avals, zero_outs = [], [], [], []
        for alloc in nc.m.functions[0].allocations:
            if not isinstance(alloc, mybir.MemoryLocationSet):
                continue
            name = alloc.memorylocations[0].name
            if alloc.kind == "ExternalInput":
                if name != partition_name:
                    in_names.append(name)
            elif alloc.kind == "ExternalOutput":
                shape = tuple(alloc.tensor_shape)
                dtype = mybir.dt.np(alloc.dtype)
                out_names.append(name)
                out_avals.append(jax.core.ShapedArray(shape, dtype))
                zero_outs.append(np.zeros(shape, dtype))
        self.param_names = list(in_names)
        self.out_names = out_names
        self.out_avals = out_avals
        self.zero_outs = zero_outs
        all_in_names = in_names + out_names
        if partition_name is not None:
            all_in_names.append(partition_name)

        def _body(*args):
            operands = list(args)
            if partition_name is not None:
                operands.append(bass2jax.partition_id_tensor())
            outs = bass2jax._bass_exec_p.bind(
                *operands,
                out_avals=tuple(out_avals),
                in_names=tuple(all_in_names),
                out_names=tuple(out_names),
                lowering_input_output_aliases=(),
                sim_require_finite=True,
                sim_require_nnan=True,
                nc=nc,
            )
            return tuple(outs)

        devices = jax.devices()[:n_cores]
        assert len(devices) == n_cores, f"need {n_cores} cores"
        self.mesh = Mesh(np.asarray(devices), ("core",))
        n_args = len(self.param_names) + len(out_names)
        self.fn = jax.jit(
            shard_map(
                _body,
                mesh=self.mesh,
                in_specs=(PartitionSpec("core"),) * n_args,
                out_specs=(PartitionSpec("core"),) * len(out_names),
                check_rep=False,
            ),
            keep_unused=True,
        )

    def concat_args(self, in_maps):
        args = [
            np.concatenate([m[name] for m in in_maps], axis=0)
            for name in self.param_names
        ]
        args += [
            np.concatenate([z] * self.n_cores, axis=0) for z in self.zero_outs
        ]
        return args

    def device_put_args(self, args):
        sh = NamedSharding(self.mesh, PartitionSpec("core"))
        return [jax.device_put(a, sh) for a in args]

    def run(self, args):
        outs = self.fn(*args)
        return [
            {
                name: np.asarray(outs[i]).reshape(
                    self.n_cores, *self.out_avals[i].shape
                )[c]
                for i, name in enumerate(self.out_names)
            }
            for c in range(self.n_cores)
        ]


_CACHE = {}


def _get_runner(**kw):
    key = tuple(sorted(kw.items()))
    if key not in _CACHE:
        _CACHE[key] = _Runner(build_nc(**kw), kw.get("ncores", NCORES))
    return _CACHE[key]


def _shard_inputs(X, adj, Ws, bs, n_nodes, ncores, adj_bf16):
    b = n_nodes // ncores
    cnp = ml_dtypes.bfloat16 if adj_bf16 else np.float32
    shared = {}
    for l in range(3):
        W_rel, b_rel, W_root = Ws[l]
        shared[f"wrel{l}t"] = np.ascontiguousarray(W_rel.T, dtype=np.float32)
        shared[f"wroot{l}t"] = np.ascontiguousarray(W_root.T, dtype=np.float32)
        shared[f"b{l}"] = np.ascontiguousarray(
            np.asarray(b_rel, dtype=np.float32).reshape(-1, 1)
        )
    in_maps = []
    for d in range(ncores):
        m = dict(shared)
        m["adjc"] = np.ascontiguousarray(adj[:, d * b:(d + 1) * b]).astype(cnp)
        m["xt"] = np.ascontiguousarray(X[d * b:(d + 1) * b, :].T, dtype=np.float32)
        in_maps.append(m)
    return in_maps


def kernel(
    X, adj, W_rel0, b_rel0, W_root0, W_rel1, b_rel1, W_root1, W_rel2, b_rel2, W_root2
):
    global LAST_RESULTS
    X = np.asarray(X, dtype=np.float32)
    adj = np.asarray(adj, dtype=np.float32)
    n_nodes, f_in = X.shape
    h = np.asarray(W_rel0).shape[0]
    ncores = NCORES
    b = n_nodes // ncores

    runner = _get_runner(
        n_nodes=n_nodes, f_in=f_in, h=h, ncores=ncores, adj_bf16=ADJ_BF16
    )
    Ws = [
        (np.asarray(W_rel0), np.asarray(b_rel0), np.asarray(W_root0)),
        (np.asarray(W_rel1), np.asarray(b_rel1), np.asarray(W_root1)),
        (np.asarray(W_rel2), np.asarray(b_rel2), np.asarray(W_root2)),
    ]
    in_maps = _shard_inputs(X, adj, Ws, None, n_nodes, ncores, ADJ_BF16)
    results = runner.run(runner.concat_args(in_maps))
    LAST_RESULTS = results

    y = np.empty((n_nodes, h), dtype=np.float32)
    for d in range(ncores):
        y[d * b:(d + 1) * b, :] = results[d]["out"].T
    return y


def time_hw(inputs, iters=30, warmup=3):
    """Execute the compiled NEFF repeatedly on device-resident buffers.

    Returns (median_s, min_s, all_times). Includes axon dispatch RTT, so
    treat the minimum as an upper bound on HW exec time.
    """
    import time as _t

    X = np.asarray(inputs["X"], np.float32)
    adj = np.asarray(inputs["adj"], np.float32)
    n_nodes, f_in = X.shape
    h = np.asarray(inputs["W_rel0"]).shape[0]
    runner = _get_runner(
        n_nodes=n_nodes, f_in=f_in, h=h, ncores=NCORES, adj_bf16=ADJ_BF16
    )
    Ws = [
        (inputs["W_rel0"], inputs["b_rel0"], inputs["W_root0"]),
        (inputs["W_rel1"], inputs["b_rel1"], inputs["W_root1"]),
        (inputs["W_rel2"], inputs["b_rel2"], inputs["W_root2"]),
    ]
    in_maps = _shard_inputs(X, adj, Ws, None, n_nodes, NCORES, ADJ_BF16)
    dev_args = runner.device_put_args(runner.concat_args(in_maps))
    for _ in range(warmup):
        jax.block_until_ready(runner.fn(*dev_args))
    times = []
    for _ in range(iters):
        t0 = _t.perf_counter()
        jax.block_until_ready(runner.fn(*dev_args))
        times.append(_t.perf_counter() - t0)
    times.sort()
    return times[len(times) // 2], times[0], times


# revision 21
# speedup vs baseline: 25.0008x; 25.0008x over previous
"""Trainium2 Bass kernel: 3-layer PyG-style GraphConv stack on a dense
weighted adjacency (nn_NeuroGNN_GNN_GraphConv).

    h_{l+1} = relu( (A^T h_l) @ W_rel_l^T + b_l + h_l @ W_root_l^T )

Distribution (8 NeuronCores): column-shard adj (= shard the OUTPUT node
block): core d holds adj[:, d*B:(d+1)*B] and computes the full
contraction over all 16384 source nodes for its own 2048 output nodes.
The only cross-core traffic is an AllGather of hr = h_l @ W_rel_l^T
([2048, 64] bf16 per core) at each layer boundary.

On-chip layout: h is kept transposed (hT [64 feat, 2048 nodes]) so that
every matmul lands in its natural orientation with zero on-chip
transposes:
  hr[j,f]   (psum) = lhsT(hT[:, jtile]) ^T @ W_relT          -> cast bf16
  aggT[f,i] (psum) = sum_j lhsT(hr[jtile])^T @ adj[jtile, :] (bf16 x bf16)
  aggT     +=        lhsT(W_rootT)^T @ hT[:, isub]           (root term)
  hT'       = relu(aggT + b)                                  (ScalarE)

adj is streamed from HBM as bf16 (host-converted): 64 MiB/core/layer,
which is the roofline term (memory-bound problem).
"""

import numpy as np
import ml_dtypes

import jax
from jax.experimental.shard_map import shard_map
from jax.sharding import Mesh, NamedSharding, PartitionSpec

from concourse import bacc, bass, bass2jax, mybir, tile

F32 = mybir.dt.float32
F32R = mybir.dt.float32r
BF16 = mybir.dt.bfloat16

N_NODES = 16384
F_IN = 256
H = 64
NCORES = 8

# knobs
ADJ_BF16 = True       # stream adj (and hr) as bf16; False -> fp32 (+f32r matmul)
CHUNK = 4             # j-tiles (128 rows) per adj DMA
ADJ_BUFS = 4          # in-flight adj chunks
N_CACHE = 0           # leading adj chunks kept resident in SBUF across layers
                      # (n_cache=7/bufs=3 measured slower on HW: 618us vs 522us)
TRACE = False         # set True (by test.py) to capture NTFF profile
LAST_RESULTS = None   # BassKernelResults of the most recent run


def build_nc(n_nodes=N_NODES, f_in=F_IN, h=H, ncores=NCORES, adj_bf16=ADJ_BF16,
             single=False, reps=1, chunk=None, adj_bufs=None, n_cache=None):
    """Build the SPMD Bass program (one program, runs on all 8 cores).

    single=True replaces the AllGather with a local DRAM copy so the
    program has no collectives (for TimelineSim cost modeling); shapes
    per core are unchanged.
    """
    b = n_nodes // ncores          # own output block width / own j-rows
    n_jt = n_nodes // 128          # global j-tiles
    own_jt = b // 128              # j-tiles in own block
    isub = min(512, b)             # psum bank width for the big matmul
    n_isub = b // isub
    c0 = f_in // 128               # K-tiles over input features (layer 0)
    chunk = chunk if chunk is not None else CHUNK
    adj_bufs = adj_bufs if adj_bufs is not None else ADJ_BUFS
    n_cache = n_cache if n_cache is not None else N_CACHE
    n_cache = min(n_cache, n_jt // chunk)

    cdt = BF16 if adj_bf16 else F32
    # for fp32 we bitcast the big matmul operands to float32r (full-rate fp32)
    mmdt = BF16 if adj_bf16 else F32R

    def mm(ap):
        return ap if adj_bf16 else ap.bitcast(F32R)

    nc = bacc.Bacc(
        "TRN2",
        target_bir_lowering=False,
        debug=False,
        num_devices=ncores,
    )

    adjc = nc.dram_tensor("adjc", [n_nodes, b], cdt, kind="ExternalInput").ap()
    xt = nc.dram_tensor("xt", [f_in, b], F32, kind="ExternalInput").ap()
    wrelt = [
        nc.dram_tensor("wrel0t", [f_in, h], F32, kind="ExternalInput").ap(),
        nc.dram_tensor("wrel1t", [h, h], F32, kind="ExternalInput").ap(),
        nc.dram_tensor("wrel2t", [h, h], F32, kind="ExternalInput").ap(),
    ]
    wroott = [
        nc.dram_tensor("wroot0t", [f_in, h], F32, kind="ExternalInput").ap(),
        nc.dram_tensor("wroot1t", [h, h], F32, kind="ExternalInput").ap(),
        nc.dram_tensor("wroot2t", [h, h], F32, kind="ExternalInput").ap(),
    ]
    bias = [
        nc.dram_tensor(f"b{l}", [h, 1], F32, kind="ExternalInput").ap()
        for l in range(3)
    ]
    out = nc.dram_tensor("out", [h, b], F32, kind="ExternalOutput").ap()

    rg = [list(range(ncores))]

    with tile.TileContext(nc) as tc:
        with (
            tc.tile_pool(name="const", bufs=1) as const,
            tc.tile_pool(name="adj", bufs=adj_bufs) as adjp,
            tc.tile_pool(name="resid", bufs=1) as residp,
            tc.tile_pool(name="hrf", bufs=8) as hrfp,
            tc.tile_pool(name="work", bufs=2) as work,
            tc.tile_pool(name="psb", bufs=n_isub, space="PSUM") as psb,
            tc.tile_pool(name="psh", bufs=2, space="PSUM") as psh,
            tc.tile_pool(name="dram", bufs=1, space="DRAM") as dram,
        ):
            # ---- constants ----
            xt_sb = const.tile([128, c0, b], F32)
            for c in range(c0):
                nc.sync.dma_start(xt_sb[:, c, :], xt[c * 128:(c + 1) * 128, :])

            wrel_sb = []
            wroot_sb = []
            # layer 0 weights have f_in rows -> c0 K-tiles
            w0rel = const.tile([128, c0, h], F32, name="w0rel")
            w0root = const.tile([128, c0, h], F32, name="w0root")
            for c in range(c0):
                nc.sync.dma_start(w0rel[:, c, :], wrelt[0][c * 128:(c + 1) * 128, :])
                nc.sync.dma_start(w0root[:, c, :], wroott[0][c * 128:(c + 1) * 128, :])
            wrel_sb.append(w0rel)
            wroot_sb.append(w0root)
            for l in (1, 2):
                wr = const.tile([h, h], F32, name=f"w{l}rel")
                wo = const.tile([h, h], F32, name=f"w{l}root")
                nc.sync.dma_start(wr[:], wrelt[l][:])
                nc.sync.dma_start(wo[:], wroott[l][:])
                wrel_sb.append(wr)
                wroot_sb.append(wo)
            b_sb = []
            for l in range(3):
                bt = const.tile([h, 1], F32, name=f"b{l}sb")
                nc.sync.dma_start(bt[:], bias[l][:])
                b_sb.append(bt)

            resident = {}  # chunk index -> SBUF-resident adj tile
            for rep in range(reps):
                hT = None  # current hidden state, transposed [h, b] (own block)
                for l in range(3):
                    # ---- A: local hr tiles + AllGather ----
                    hr_own = work.tile(
                        [128, own_jt, h], cdt, name="hr_own", tag="hr_own"
                    )
                    for t in range(own_jt):
                        ps = psh.tile([128, h], F32, name="ps_hr", tag="ps_hr")
                        if l == 0:
                            for c in range(c0):
                                nc.tensor.matmul(
                                    ps[:],
                                    xt_sb[:, c, t * 128:(t + 1) * 128],
                                    wrel_sb[0][:, c, :],
                                    start=(c == 0),
                                    stop=(c == c0 - 1),
                                )
                        else:
                            nc.tensor.matmul(
                                ps[:],
                                hT[:, t * 128:(t + 1) * 128],
                                wrel_sb[l][:],
                                start=True,
                                stop=True,
                            )
                        nc.vector.tensor_copy(hr_own[:, t, :], ps[:])

                    ag_in = dram.tile([b, h], cdt, name=f"ag_in{l}_{rep}")
                    nc.sync.dma_start(
                        ag_in.rearrange("(t p) f -> p t f", p=128), hr_own[:]
                    )
                    if single:
                        ag_out = dram.tile([n_nodes, h], cdt, name=f"ag_out{l}_{rep}")
                        nc.sync.dma_start(ag_out[0:b, :], ag_in[:])
                    else:
                        ag_out = dram.tile(
                            [n_nodes, h], cdt, name=f"ag_out{l}_{rep}",
                            addr_space="Shared",
                        )
                        nc.gpsimd.collective_compute(
                            "AllGather",
                            mybir.AluOpType.bypass,
                            replica_groups=rg,
                            ins=[ag_in.opt()],
                            outs=[ag_out.opt()],
                        )

                    # ---- B: big matmul  aggT[f, i] += hr^T adj (+ root) ----
                    pss = [
                        psb.tile([h, isub], F32, name=f"ps_big{s}", tag="ps_big")
                        for s in range(n_isub)
                    ]
                    for ch in range(n_jt // chunk):
                        if ch in resident:
                            adj_t = resident[ch]
                        else:
                            if ch < n_cache:
                                adj_t = residp.tile(
                                    [128, chunk, b], cdt, name=f"adj_res{ch}"
                                )
                                resident[ch] = adj_t
                            else:
                                adj_t = adjp.tile(
                                    [128, chunk, b], cdt, name="adj_t", tag="adj_t"
                                )
                            nc.sync.dma_start(
                                adj_t[:],
                                adjc[
                                    ch * chunk * 128:(ch + 1) * chunk * 128, :
                                ].rearrange("(u p) n -> p u n", p=128),
                            )
                        hrf = hrfp.tile([128, chunk, h], cdt, name="hrf", tag="hrf")
                        nc.sync.dma_start(
                            hrf[:],
                            ag_out[
                                ch * chunk * 128:(ch + 1) * chunk * 128, :
                            ].rearrange("(q p) f -> p q f", p=128),
                        )
                        for u in range(chunk):
                            g = ch * chunk + u
                            for s in range(n_isub):
                                nc.tensor.matmul(
                                    pss[s][:],
                                    mm(hrf[:, u, :]),
                                    mm(adj_t[:, u, s * isub:(s + 1) * isub]),
                                    start=(g == 0),
                                    stop=False,
                                )
                    # root term, accumulated into the same psum banks
                    for s in range(n_isub):
                        if l == 0:
                            for c in range(c0):
                                nc.tensor.matmul(
                                    pss[s][:],
                                    wroot_sb[0][:, c, :],
                                    xt_sb[:, c, s * isub:(s + 1) * isub],
                                    start=False,
                                    stop=(c == c0 - 1),
                                )
                        else:
                            nc.tensor.matmul(
                                pss[s][:],
                                wroot_sb[l][:],
                                hT[:, s * isub:(s + 1) * isub],
                                start=False,
                                stop=True,
                            )

                    # ---- C: hT' = relu(aggT + b) ----
                    hT_new = work.tile([h, b], F32, name="hT", tag="hT")
                    for s in range(n_isub):
                        nc.scalar.activation(
                            hT_new[:, s * isub:(s + 1) * isub],
                            pss[s][:],
                            mybir.ActivationFunctionType.Relu,
                            bias=b_sb[l][:],
                        )
                    hT = hT_new

                nc.sync.dma_start(out[:], hT[:])

    nc.compile()
    return nc


class _Runner:
    """Persistent PJRT executor for one compiled Bass program.

    Mirrors concourse.bass2jax.run_bass_via_pjrt but keeps the jitted
    callable (and optionally device-resident inputs) alive across calls,
    and skips output-buffer donation so the same buffers can be re-executed
    for timing. The kernel writes every element of its outputs, so the
    zero-init the native path provides is not load-bearing here.
    """

    def __init__(self, nc, n_cores):
        bass2jax.install_neuronx_cc_hook()
        assert nc.dbg_addr is None, "build with debug=False"
        self.nc = nc
        self.n_cores = n_cores
        partition_name = (
            nc.partition_id_tensor.name if nc.partition_id_tensor else None
        )
        in_names, out_names, out_avals, zero_outs = [], [], [], []
        for alloc in nc.m.functions[0].allocations:
            if not isinstance(alloc, mybir.MemoryLocationSet):
                continue
            name = alloc.memorylocations[0].name
            if alloc.kind == "ExternalInput":
                if name != partition_name:
                    in_names.append(name)
            elif alloc.kind == "ExternalOutput":
                shape = tuple(alloc.tensor_shape)
                dtype = mybir.dt.np(alloc.dtype)
                out_names.append(name)
                out_avals.append(jax.core.ShapedArray(shape, dtype))
                zero_outs.append(np.zeros(shape, dtype))
        self.param_names = list(in_names)
        self.out_names = out_names
        self.out_avals = out_avals
        self.zero_outs = zero_outs
        all_in_names = in_names + out_names
        if partition_name is not None:
            all_in_names.append(partition_name)

        def _body(*args):
            operands = list(args)
            if partition_name is not None:
                operands.append(bass2jax.partition_id_tensor())
            outs = bass2jax._bass_exec_p.bind(
                *operands,
                out_avals=tuple(out_avals),
                in_names=tuple(all_in_names),
                out_names=tuple(out_names),
                lowering_input_output_aliases=(),
                sim_require_finite=True,
                sim_require_nnan=True,
                nc=nc,
            )
            return tuple(outs)

        devices = jax.devices()[:n_cores]
        assert len(devices) == n_cores, f"need {n_cores} cores"
        self.mesh = Mesh(np.asarray(devices), ("core",))
        n_args = len(self.param_names) + len(out_names)
        self.fn = jax.jit(
            shard_map(
                _body,
                mesh=self.mesh,
                in_specs=(PartitionSpec("core"),) * n_args,
                out_specs=(PartitionSpec("core"),) * len(out_names),
                check_rep=False,
            ),
            keep_unused=True,
        )

    def concat_args(self, in_maps):
        args = [
            np.concatenate([m[name] for m in in_maps], axis=0)
            for name in self.param_names
        ]
        args += [
            np.concatenate([z] * self.n_cores, axis=0) for z in self.zero_outs
        ]
        return args

    def device_put_args(self, args):
        sh = NamedSharding(self.mesh, PartitionSpec("core"))
        return [jax.device_put(a, sh) for a in args]

    def run(self, args):
        outs = self.fn(*args)
        return [
            {
                name: np.asarray(outs[i]).reshape(
                    self.n_cores, *self.out_avals[i].shape
                )[c]
                for i, name in enumerate(self.out_names)
            }
            for c in range(self.n_cores)
        ]


_CACHE = {}


def _get_runner(**kw):
    key = tuple(sorted(kw.items()))
    if key not in _CACHE:
        _CACHE[key] = _Runner(build_nc(**kw), kw.get("ncores", NCORES))
    return _CACHE[key]


def _shard_inputs(X, adj, Ws, bs, n_nodes, ncores, adj_bf16):
    b = n_nodes // ncores
    cnp = ml_dtypes.bfloat16 if adj_bf16 else np.float32
    shared = {}
    for l in range(3):
        W_rel, b_rel, W_root = Ws[l]
        shared[f"wrel{l}t"] = np.ascontiguousarray(W_rel.T, dtype=np.float32)
        shared[f"wroot{l}t"] = np.ascontiguousarray(W_root.T, dtype=np.float32)
        shared[f"b{l}"] = np.ascontiguousarray(
            np.asarray(b_rel, dtype=np.float32).reshape(-1, 1)
        )
    in_maps = []
    for d in range(ncores):
        m = dict(shared)
        m["adjc"] = np.ascontiguousarray(adj[:, d * b:(d + 1) * b]).astype(cnp)
        m["xt"] = np.ascontiguousarray(X[d * b:(d + 1) * b, :].T, dtype=np.float32)
        in_maps.append(m)
    return in_maps


def kernel(
    X, adj, W_rel0, b_rel0, W_root0, W_rel1, b_rel1, W_root1, W_rel2, b_rel2, W_root2
):
    global LAST_RESULTS
    X = np.asarray(X, dtype=np.float32)
    adj = np.asarray(adj, dtype=np.float32)
    n_nodes, f_in = X.shape
    h = np.asarray(W_rel0).shape[0]
    ncores = NCORES
    b = n_nodes // ncores

    runner = _get_runner(
        n_nodes=n_nodes, f_in=f_in, h=h, ncores=ncores, adj_bf16=ADJ_BF16
    )
    Ws = [
        (np.asarray(W_rel0), np.asarray(b_rel0), np.asarray(W_root0)),
        (np.asarray(W_rel1), np.asarray(b_rel1), np.asarray(W_root1)),
        (np.asarray(W_rel2), np.asarray(b_rel2), np.asarray(W_root2)),
    ]
    in_maps = _shard_inputs(X, adj, Ws, None, n_nodes, ncores, ADJ_BF16)
    results = runner.run(runner.concat_args(in_maps))
    LAST_RESULTS = results

    y = np.empty((n_nodes, h), dtype=np.float32)
    for d in range(ncores):
        y[d * b:(d + 1) * b, :] = results[d]["out"].T
    return y


def time_hw(inputs, reps_timing=False):
    """Estimate per-execution HW time of the compiled NEFF.

    The axon PJRT tunnel adds ~90 ms fixed dispatch overhead per blocking
    round-trip, so single-execute wall time is meaningless. Two methods:

    - default: async-submit batches of sizes 8 and 40, block once per
      batch; the batch-size marginal cancels the fixed overhead but still
      includes the per-dispatch tunnel cost (upper bound on HW exec).
    - reps_timing=True: also build a NEFF with the whole computation
      unrolled 4x and use the reps marginal — cancels everything except
      true HW execution (costs one extra NEFF compile).
    """
    import time as _t

    X = np.asarray(inputs["X"], np.float32)
    adj = np.asarray(inputs["adj"], np.float32)
    n_nodes, f_in = X.shape
    h = np.asarray(inputs["W_rel0"]).shape[0]
    Ws = [
        (inputs["W_rel0"], inputs["b_rel0"], inputs["W_root0"]),
        (inputs["W_rel1"], inputs["b_rel1"], inputs["W_root1"]),
        (inputs["W_rel2"], inputs["b_rel2"], inputs["W_root2"]),
    ]
    in_maps = _shard_inputs(X, adj, Ws, None, n_nodes, NCORES, ADJ_BF16)

    def bench(runner, batch, rounds=4):
        dev = runner.device_put_args(runner.concat_args(in_maps))
        jax.block_until_ready(runner.fn(*dev))
        best = None
        for _ in range(rounds):
            t0 = _t.perf_counter()
            outs = [runner.fn(*dev) for _ in range(batch)]
            jax.block_until_ready(outs)
            dt = _t.perf_counter() - t0
            best = dt if best is None else min(best, dt)
        return best

    kw = dict(n_nodes=n_nodes, f_in=f_in, h=h, ncores=NCORES, adj_bf16=ADJ_BF16)
    r1 = _get_runner(**kw)
    if reps_timing:
        r4 = _get_runner(**kw, reps=4)
        t1 = bench(r1, 16)
        t4 = bench(r4, 16)
        return (t4 - t1) / (16 * 3)
    ta = bench(r1, 8)
    tb = bench(r1, 40)
    return (tb - ta) / 32
